# revision 33
# baseline (speedup 1.0000x reference)
"""SSIM loss kernel for Trainium2, SPMD over 8 NeuronCores.

Inputs: img1, img2 [16,3,512,512] f32. Output: scalar mean SSIM (f32).
Sharding: batch dim 16 -> 2 per core; host sums per-core partial sums.

Math (per pixel, 11x11 Gaussian window, C1=1e-4, C2=9e-4), u/v basis:
  u = x+y, v = x-y; streams u, v, p=u^2/2, q=v^2/2
  ma = conv(u) = mu1+mu2, mb = conv(v) = mu1-mu2
  cdm = conv(p) - conv(q) = 2*conv(xy) = cd
  csm = conv(p) + conv(q) = conv(x^2)+conv(y^2) = cs
  A = ma^2/2, B = mb^2/2; r2c1 = (A+C1)-B = 2 mu1 mu2 + C1
  mqc1 = (A+C1)+B = mu1^2+mu2^2 + C1
  num = (cdm - r2c1 + C1+C2) * r2c1   [custom DVE nd2]
  den = (csm - mqc1 + C1+C2) * mqc1   [custom DVE nd2]
  ssim = num * recip(den); mean via PE ones-matmul reduction.

Implementation notes:
- Inputs loaded fp32 by hardware DMA (sync + scalar HWDGE queues, one
  tensor each), cast to fp16 by ACT copies; u/v adds on DVE fp16 TT (2x).
- 11-tap separable conv as dense band matmuls: pass 1 convolves H
  data-stationary (output transposed, W on partitions; 5 overlapping
  128-row windows, stride 96); pass 2 convolves W band-stationary.
- Pass-1 emission stream-major with per-stream PSUM->SBUF drain
  (ACT/DVE alternating); PSUM: pass1 rotates 3 banks, pass2 4, acc 1.
- Per-pixel SSIM summed on PE: acc_ps[1,512] += ones^T @ scr, one
  accumulation group across the whole kernel; host sums the 512 lanes.
- fp16 band taps ulp-nudged so sum(fl16(g)) == 1 (sigma paths are
  first-order sensitive to the weight sum).
- Software-pipelined emission (2 window-slot stagger) keeps PE warm.
"""

import math

import numpy as np

from concourse import bacc, bass, mybir, tile
from concourse.bass_utils import run_bass_kernel_spmd

B_FULL, C, H, W = 16, 3, 512, 512
N_CORES = 8
B_LOCAL = B_FULL // N_CORES          # 2
N_PLANES = B_LOCAL * C               # 6 spatial planes per core
KSZ = 11
PAD = KSZ // 2
SSIM_C1 = 0.01 ** 2
SSIM_C2 = 0.03 ** 2

STRIDE = 96
NWIN = 5
CHUNKS = [(0, 101), (101, 96), (197, 96), (293, 96), (389, 123)]
WF = NWIN * 512                      # 2560 free cols per plane

FP32 = mybir.dt.float32
FP16 = mybir.dt.float16

_OPS = {}


def _register_custom_ops():
    """Idempotently register the SSIM custom DVE ops."""
    global _OPS
    if _OPS:
        return _OPS
    import concourse.dve_ops as D
    from concourse.dve_spec import Spec, Src0, Src1, C0, C1, lower, _has_src1
    from concourse.dve_uop import DveOpSpec

    def reg(op):
        D.OPS.append(op)
        D._SUB_OPCODE_FOR_NAME[op.name] = D._CUSTOM_DVE_ROW_BASE + len(D.OPS) - 1
        D.CUSTOM_DVE_SPECS[op.name] = op.spec
        for ver in ("v3", "v4"):
            uops = lower(op.spec, ver=ver)
            so = DveOpSpec(name=op.name, opcode=D.get_dve_sub_opcode(op.name),
                           uops=uops, rd1_en=_has_src1(op.spec))
            op.uops_sha[ver] = so.sha(ver)
        return op

    if "SSIM_ND_ANT" in D._SUB_OPCODE_FOR_NAME:
        nd = next(o for o in D.OPS if o.name == "SSIM_ND_ANT")
    else:
        nd = reg(D.DveOp(
            "SSIM_ND_ANT",
            Spec(body=(Src0 - Src1 + C0) * (Src1 + C1),
                 reference=lambda in0, in1, s0, s1, imm2:
                     (in0.astype(np.float32) - in1 + s0)
                     * (in1.astype(np.float32) + s1)),
            subdim=False, uops_sha={}))
    _OPS = {"nd": nd,
            "recip": D.RECIPROCAL_APPROX_FAST,
            "recip_consts": D.RECIP_APPROX_FAST_CONSTS}
    return _OPS


def _gaussian_1d():
    x = np.arange(KSZ)
    g = np.exp(-((x - KSZ // 2) ** 2) / (2.0 * 1.5 ** 2))
    return (g / g.sum()).astype(np.float64)


def _gaussian_1d_f16():
    """fp16 taps nudged by +-1 ulp so sum(fl16(g)) == 1 to ~1e-7."""
    g16 = _gaussian_1d().astype(np.float16)
    for _ in range(200):
        e = g16.astype(np.float64).sum() - 1.0
        if abs(e) < 5e-8:
            break
        best = None
        for i in range(KSZ):
            step = np.nextafter(g16[i], np.float16(1.0 if e < 0 else 0.0))
            ne = e + (float(step) - float(g16[i]))
            if best is None or abs(ne) < abs(best[1]):
                best = (i, ne, step)
        i, ne, step = best
        if abs(ne) >= abs(e):
            break
        g16[i] = step
    return g16.astype(np.float64)


def _build_bands():
    """[128, 5*128] f16; window c at cols [128c, 128c+n_c).
    out[s+jj] = sum_r band[r, 128c+jj] * x[96c + r]."""
    g = _gaussian_1d_f16()
    bands = np.zeros((128, NWIN * 128), dtype=np.float64)
    for c, (s, n) in enumerate(CHUNKS):
        r0 = STRIDE * c
        for r in range(128):
            for jj in range(n):
                t = (r0 + r) - (s + jj) + PAD
                if 0 <= t < KSZ:
                    bands[r, c * 128 + jj] = g[t]
    return bands.astype(np.float16)


def _build_graph():
    ops = _register_custom_ops()
    nc = bacc.Bacc()
    img1 = nc.declare_dram_parameter("img1", [B_LOCAL, C, H, W], FP16, isOutput=False)
    img2 = nc.declare_dram_parameter("img2", [B_LOCAL, C, H, W], FP16, isOutput=False)
    bands = nc.declare_dram_parameter("bands", [128, NWIN * 128], FP16, isOutput=False)
    bandsn = nc.declare_dram_parameter("bandsn", [128, NWIN * 128], FP16, isOutput=False)
    bands2 = nc.declare_dram_parameter("bands2", [128, NWIN * 128], FP16, isOutput=False)
    out = nc.declare_dram_parameter("out", [1, 512], FP32, isOutput=True)

    Alu = mybir.AluOpType
    Act = mybir.ActivationFunctionType
    rc = ops["recip_consts"]
    INV_SQRT2 = 1.0 / math.sqrt(2.0)
    C12 = SSIM_C1 + SSIM_C2
    QS = {"x": 0, "y": 1, "w": 2, "s": 3}   # stream order in yv blocks
    N_ACC = N_PLANES * NWIN                 # total acc matmuls (30)

    with tile.TileContext(nc) as tc:
        with (
            tc.tile_pool(name="const_p", bufs=1) as const_p,
            tc.tile_pool(name="in16_p", bufs=2) as in16_p,
            tc.tile_pool(name="pre_p", bufs=2) as pre_p,
            tc.tile_pool(name="yv_p", bufs=2) as yv_p,
            tc.tile_pool(name="post_p", bufs=2) as post_p,
            tc.tile_pool(name="ps1_p", bufs=1, space="PSUM") as ps1_p,
            tc.tile_pool(name="ps2_p", bufs=1, space="PSUM") as ps2_p,
            tc.tile_pool(name="acc_p", bufs=1, space="PSUM") as acc_p,
        ):
            band_t = const_p.tile([128, NWIN * 128], FP16, name="band_t",
                                  tag="band_t")
            bandn_t = const_p.tile([128, NWIN * 128], FP16, name="bandn_t",
                                   tag="bandn_t")
            band2_t = const_p.tile([128, NWIN * 128], FP16, name="band2_t",
                                   tag="band2_t")
            nc.sync.dma_start(out=band_t[:], in_=bands[:, :])
            nc.sync.dma_start(out=bandn_t[:], in_=bandsn[:, :])
            nc.sync.dma_start(out=band2_t[:], in_=bands2[:, :])

            ones_t = const_p.tile([128, 1], FP16, name="ones_t", tag="ones_t")
            nc.vector.memset(ones_t[:], 1.0)

            acc_ps = acc_p.tile([1, 512], FP32, name="acc_ps")

            # --- fp16 input loads (hardware DMA), one per plane/tensor,
            # issued ~two planes ahead from the pipeline loop ---
            x16 = {}
            y16 = {}

            def emit_load(p, half=None):
                if half in (None, 0):
                    x16[p] = in16_p.tile([128, WF], FP16, name="x16",
                                         tag="x16")
                    y16[p] = in16_p.tile([128, WF], FP16, name="y16",
                                         tag="y16")
                if half is None:
                    w0, wn = 0, W
                else:
                    w0, wn = half * (W // 2), W // 2
                for dst, src, eng in ((x16[p], img1, nc.sync),
                                      (y16[p], img2, nc.scalar)):
                    ap = bass.AP(src, p * H * W + w0,
                                 [[W, 128], [STRIDE * W, NWIN], [1, wn]])
                    eng.dma_start(
                        out=dst.rearrange("pt (c w) -> pt c w",
                                          c=NWIN)[:, :, w0:w0 + wn],
                        in_=ap)

            pre = {}     # plane -> dict of u/v/p/q stream tiles (fp16)

            def emit_pre(p, half=None):
                if half in (None, 0):
                    w16 = pre_p.tile([128, WF], FP16, name="w16", tag="w16")
                    sx16 = pre_p.tile([128, WF], FP16, name="sx16",
                                      tag="sx16")
                    sy16 = pre_p.tile([128, WF], FP16, name="sy16",
                                      tag="sy16")
                    pre[p] = {"w": w16, "sx": sx16, "sy": sy16}
                w16 = pre[p]["w"]
                sx16 = pre[p]["sx"]
                sy16 = pre[p]["sy"]
                if half is None:
                    sl = [slice(None)]
                else:
                    w0 = half * (W // 2)
                    sl3 = (slice(None), slice(None), slice(w0, w0 + W // 2))
                xv = x16[p].rearrange("pt (c w) -> pt c w", c=NWIN)
                yv_ = y16[p].rearrange("pt (c w) -> pt c w", c=NWIN)
                wv = w16.rearrange("pt (c w) -> pt c w", c=NWIN)
                sxv = sx16.rearrange("pt (c w) -> pt c w", c=NWIN)
                syv = sy16.rearrange("pt (c w) -> pt c w", c=NWIN)
                if half is None:
                    nc.vector.tensor_tensor(w16[:], x16[p][:], y16[p][:],
                                            Alu.mult)
                    nc.vector.tensor_tensor(sx16[:], x16[p][:], x16[p][:],
                                            Alu.mult)
                    nc.vector.tensor_tensor(sy16[:], y16[p][:], y16[p][:],
                                            Alu.mult)
                else:
                    nc.vector.tensor_tensor(wv[sl3], xv[sl3], yv_[sl3],
                                            Alu.mult)
                    nc.vector.tensor_tensor(sxv[sl3], xv[sl3], xv[sl3],
                                            Alu.mult)
                    nc.vector.tensor_tensor(syv[sl3], yv_[sl3], yv_[sl3],
                                            Alu.mult)

            yv = {}      # plane -> [128, 5*2048] f16 (per cw: u|v|p|q 512-blocks)

            def emit_pass1(p, cw):
                if cw == 0:
                    yv[p] = yv_p.tile([128, NWIN * 2048], FP16, name="yv",
                                      tag="yv")
                pr = pre[p]
                # stream-major: each stream's matmuls then its drain copy,
                # rotating over 3 PSUM banks.
                dve_drains = ()
                srcs = {"x": (x16[p],), "y": (y16[p],), "w": (pr["w"],),
                        "s": (pr["sx"], pr["sy"])}
                for qn in ("x", "y", "w", "s"):
                    p1 = ps1_p.tile([128, 512], FP32, name=f"p1{qn}", tag="p1",
                                    bufs=3)
                    bsrc = band2_t if qn == "w" else band_t
                    for c, (s, n) in enumerate(CHUNKS):
                        col = c * 512 + STRIDE * cw
                        bnd = bsrc[:, c * 128:c * 128 + n]
                        parts = srcs[qn]
                        for pi, ps in enumerate(parts):
                            nc.tensor.matmul(
                                p1[:, s:s + n], ps[:, col:col + 128],
                                bnd, start=(pi == 0),
                                stop=(pi == len(parts) - 1))
                    dst = yv[p][:, cw * 2048 + QS[qn] * 512:
                                cw * 2048 + QS[qn] * 512 + 512]
                    if qn in dve_drains:
                        nc.vector.tensor_copy(dst, p1[:, :])
                    else:
                        nc.scalar.copy(dst, p1[:, :])

            acc_n = [0]

            def emit_pass2_post(p, c2):
                s2, n2 = CHUNKS[c2]
                bnd = band_t[:, c2 * 128:c2 * 128 + n2]
                bndn = bandn_t[:, c2 * 128:c2 * 128 + n2]
                mamb = ps2_p.tile([128, 1024], FP32, name="mamb", tag="mamb")
                cdcs = ps2_p.tile([128, 1024], FP32, name="cdcs", tag="cdcs")
                cdm = cdcs[:, 0:512]
                csm = cdcs[:, 512:1024]
                yvx = yv[p][:, c2 * 2048 + QS["x"] * 512:c2 * 2048 + QS["x"] * 512 + 512]
                yvy = yv[p][:, c2 * 2048 + QS["y"] * 512:c2 * 2048 + QS["y"] * 512 + 512]
                yvw = yv[p][:, c2 * 2048 + QS["w"] * 512:c2 * 2048 + QS["w"] * 512 + 512]
                yvs = yv[p][:, c2 * 2048 + QS["s"] * 512:c2 * 2048 + QS["s"] * 512 + 512]
                nc.tensor.matmul(mamb[:n2, 0:512], bnd, yvx, start=True, stop=False)
                nc.tensor.matmul(mamb[:n2, 0:512], bnd, yvy, start=False, stop=True)
                nc.tensor.matmul(mamb[:n2, 512:1024], bnd, yvx, start=True, stop=False)
                nc.tensor.matmul(mamb[:n2, 512:1024], bndn, yvy, start=False, stop=True)
                nc.tensor.matmul(cdcs[:n2, 0:512], bnd, yvw, start=True, stop=True)
                nc.tensor.matmul(cdcs[:n2, 512:1024], bnd, yvs, start=True, stop=True)

                ABt = post_p.tile([128, 1024], FP16, name="ABt", tag="ABt")
                rst = post_p.tile([128, 1024], FP16, name="rst", tag="rst")
                ndt = post_p.tile([128, 1024], FP16, name="ndt", tag="ndt")
                rect = post_p.tile([128, 512], FP16, name="rect", tag="rect")
                scr = post_p.tile([128, 512], FP16, name="scr", tag="scr",
                                  bufs=3)
                nc.scalar.activation(ABt[:n2, :], mamb[:n2, :], Act.Square,
                                     scale=INV_SQRT2)
                nc.vector.tensor_tensor(
                    rst[:n2, 0:512], ABt[:n2, 0:512], ABt[:n2, 512:1024],
                    Alu.subtract)
                nc.vector.tensor_tensor(
                    rst[:n2, 512:1024], ABt[:n2, 0:512], ABt[:n2, 512:1024],
                    Alu.add)
                nc.vector._custom_dve(
                    ops["nd"], out=ndt[:n2, :], in0=cdcs[:n2, :],
                    in1=rst[:n2, :], s0=SSIM_C2, s1=SSIM_C1)
                nc.vector._custom_dve(
                    ops["recip"], out=rect[:n2, :], in0=ndt[:n2, 512:1024],
                    s0=rc["s0"], s1=rc["s1"], imm2=rc["imm2"])
                nc.vector.tensor_tensor(
                    scr[:n2, :], ndt[:n2, 0:512], rect[:n2, :], Alu.mult)
                acc_q.append((scr, n2))

            def emit_acc():
                scr, n2 = acc_q.popleft()
                k = acc_n[0]
                acc_n[0] += 1
                nc.tensor.matmul(acc_ps[:, :], ones_t[:n2, :], scr[:n2, :],
                                 start=(k == 0), stop=(k == N_ACC - 1))

            from collections import deque
            pending = deque()
            acc_q = deque()
            emit_load(0)
            emit_load(1)
            for p in range(N_PLANES):
                for cw in range(NWIN):
                    if p == 0 and cw == 0:
                        emit_pre(0)
                    emit_pass1(p, cw)
                    pending.append((p, cw))
                    lag = 2 if p == N_PLANES - 1 and cw >= 3 else 2
                    if len(pending) > lag:
                        emit_pass2_post(*pending.popleft())
                    if len(acc_q) > 1:
                        emit_acc()
                    if cw == 1 and p + 2 < N_PLANES:
                        emit_load(p + 2)
                    if cw == 1 and p + 1 < N_PLANES:
                        emit_pre(p + 1)
            while pending:
                emit_pass2_post(*pending.popleft())
            while acc_q:
                emit_acc()

            out_sb = const_p.tile([1, 512], FP32, name="out_sb", tag="out_sb")
            nc.vector.tensor_copy(out_sb[:, :], acc_ps[:, :])
            nc.sync.dma_start(out=out[:, :], in_=out_sb[:, :])

    nc.compile()
    return nc


_NC_CACHE = None


def _in_maps(img1, img2):
    img1 = np.ascontiguousarray(img1, dtype=np.float32).astype(np.float16)
    img2 = np.ascontiguousarray(img2, dtype=np.float32).astype(np.float16)
    bands = _build_bands()
    return [
        {
            "img1": img1[i * B_LOCAL:(i + 1) * B_LOCAL],
            "img2": img2[i * B_LOCAL:(i + 1) * B_LOCAL],
            "bands": bands,
            "bandsn": (-bands.astype(np.float32)).astype(np.float16),
            "bands2": (bands.astype(np.float32) * 2.0).astype(np.float16),
        }
        for i in range(N_CORES)
    ]


def kernel(img1: np.ndarray, img2: np.ndarray) -> np.ndarray:
    global _NC_CACHE
    if _NC_CACHE is None:
        _NC_CACHE = _build_graph()
    nc = _NC_CACHE

    res = run_bass_kernel_spmd(nc, _in_maps(img1, img2), list(range(N_CORES)))
    total = np.float64(0.0)
    for r in res.results:
        total += np.asarray(r["out"], dtype=np.float64).sum()
    mean = total / (B_FULL * C * H * W)
    return np.array(mean, dtype=np.float32)


# revision 34
# speedup vs baseline: 1.0344x; 1.0344x over previous
"""SSIM loss kernel for Trainium2, SPMD over 8 NeuronCores.

Inputs: img1, img2 [16,3,512,512] f32. Output: scalar mean SSIM (f32).
Sharding: batch dim 16 -> 2 per core; host sums per-core partial sums.

Math (per pixel, 11x11 Gaussian window, C1=1e-4, C2=9e-4), u/v basis:
  u = x+y, v = x-y; streams u, v, p=u^2/2, q=v^2/2
  ma = conv(u) = mu1+mu2, mb = conv(v) = mu1-mu2
  cdm = conv(p) - conv(q) = 2*conv(xy) = cd
  csm = conv(p) + conv(q) = conv(x^2)+conv(y^2) = cs
  A = ma^2/2, B = mb^2/2; r2c1 = (A+C1)-B = 2 mu1 mu2 + C1
  mqc1 = (A+C1)+B = mu1^2+mu2^2 + C1
  num = (cdm - r2c1 + C1+C2) * r2c1   [custom DVE nd2]
  den = (csm - mqc1 + C1+C2) * mqc1   [custom DVE nd2]
  ssim = num * recip(den); mean via PE ones-matmul reduction.

Implementation notes:
- Inputs loaded fp32 by hardware DMA (sync + scalar HWDGE queues, one
  tensor each), cast to fp16 by ACT copies; u/v adds on DVE fp16 TT (2x).
- 11-tap separable conv as dense band matmuls: pass 1 convolves H
  data-stationary (output transposed, W on partitions; 5 overlapping
  128-row windows, stride 96); pass 2 convolves W band-stationary.
- Pass-1 emission stream-major with per-stream PSUM->SBUF drain
  (ACT/DVE alternating); PSUM: pass1 rotates 3 banks, pass2 4, acc 1.
- Per-pixel SSIM summed on PE: acc_ps[1,512] += ones^T @ scr, one
  accumulation group across the whole kernel; host sums the 512 lanes.
- fp16 band taps ulp-nudged so sum(fl16(g)) == 1 (sigma paths are
  first-order sensitive to the weight sum).
- Software-pipelined emission (2 window-slot stagger) keeps PE warm.
"""

import math

import numpy as np

from concourse import bacc, bass, mybir, tile
from concourse.bass_utils import run_bass_kernel_spmd

B_FULL, C, H, W = 16, 3, 512, 512
N_CORES = 8
B_LOCAL = B_FULL // N_CORES          # 2
N_PLANES = B_LOCAL * C               # 6 spatial planes per core
KSZ = 11
PAD = KSZ // 2
SSIM_C1 = 0.01 ** 2
SSIM_C2 = 0.03 ** 2

STRIDE = 96
NWIN = 5
CHUNKS = [(0, 101), (101, 96), (197, 96), (293, 96), (389, 123)]
WF = NWIN * 512                      # 2560 free cols per plane

FP32 = mybir.dt.float32
FP16 = mybir.dt.float16

_OPS = {}


def _register_custom_ops():
    """Idempotently register the SSIM custom DVE ops."""
    global _OPS
    if _OPS:
        return _OPS
    import concourse.dve_ops as D
    from concourse.dve_spec import Spec, Src0, Src1, C0, C1, lower, _has_src1
    from concourse.dve_uop import DveOpSpec

    def reg(op):
        D.OPS.append(op)
        D._SUB_OPCODE_FOR_NAME[op.name] = D._CUSTOM_DVE_ROW_BASE + len(D.OPS) - 1
        D.CUSTOM_DVE_SPECS[op.name] = op.spec
        for ver in ("v3", "v4"):
            uops = lower(op.spec, ver=ver)
            so = DveOpSpec(name=op.name, opcode=D.get_dve_sub_opcode(op.name),
                           uops=uops, rd1_en=_has_src1(op.spec))
            op.uops_sha[ver] = so.sha(ver)
        return op

    if "SSIM_ND_ANT" in D._SUB_OPCODE_FOR_NAME:
        nd = next(o for o in D.OPS if o.name == "SSIM_ND_ANT")
    else:
        nd = reg(D.DveOp(
            "SSIM_ND_ANT",
            Spec(body=(Src0 - Src1 + C0) * (Src1 + C1),
                 reference=lambda in0, in1, s0, s1, imm2:
                     (in0.astype(np.float32) - in1 + s0)
                     * (in1.astype(np.float32) + s1)),
            subdim=False, uops_sha={}))
    _OPS = {"nd": nd,
            "recip": D.RECIPROCAL_APPROX_FAST,
            "recip_consts": D.RECIP_APPROX_FAST_CONSTS}
    return _OPS


def _gaussian_1d():
    x = np.arange(KSZ)
    g = np.exp(-((x - KSZ // 2) ** 2) / (2.0 * 1.5 ** 2))
    return (g / g.sum()).astype(np.float64)


def _gaussian_1d_f16():
    """fp16 taps nudged by +-1 ulp so sum(fl16(g)) == 1 to ~1e-7."""
    g16 = _gaussian_1d().astype(np.float16)
    for _ in range(200):
        e = g16.astype(np.float64).sum() - 1.0
        if abs(e) < 5e-8:
            break
        best = None
        for i in range(KSZ):
            step = np.nextafter(g16[i], np.float16(1.0 if e < 0 else 0.0))
            ne = e + (float(step) - float(g16[i]))
            if best is None or abs(ne) < abs(best[1]):
                best = (i, ne, step)
        i, ne, step = best
        if abs(ne) >= abs(e):
            break
        g16[i] = step
    return g16.astype(np.float64)


def _build_bands():
    """[128, 5*128] f16; window c at cols [128c, 128c+n_c).
    out[s+jj] = sum_r band[r, 128c+jj] * x[96c + r]."""
    g = _gaussian_1d_f16()
    bands = np.zeros((128, NWIN * 128), dtype=np.float64)
    for c, (s, n) in enumerate(CHUNKS):
        r0 = STRIDE * c
        for r in range(128):
            for jj in range(n):
                t = (r0 + r) - (s + jj) + PAD
                if 0 <= t < KSZ:
                    bands[r, c * 128 + jj] = g[t]
    return bands.astype(np.float16)


def _build_graph():
    ops = _register_custom_ops()
    nc = bacc.Bacc()
    img1 = nc.declare_dram_parameter("img1", [B_LOCAL, C, H, W], FP16, isOutput=False)
    img2 = nc.declare_dram_parameter("img2", [B_LOCAL, C, H, W], FP16, isOutput=False)
    bands = nc.declare_dram_parameter("bands", [128, NWIN * 128], FP16, isOutput=False)
    bandsn = nc.declare_dram_parameter("bandsn", [128, NWIN * 128], FP16, isOutput=False)
    bands2 = nc.declare_dram_parameter("bands2", [128, NWIN * 128], FP16, isOutput=False)
    out = nc.declare_dram_parameter("out", [1, 512], FP32, isOutput=True)

    Alu = mybir.AluOpType
    Act = mybir.ActivationFunctionType
    rc = ops["recip_consts"]
    INV_SQRT2 = 1.0 / math.sqrt(2.0)
    C12 = SSIM_C1 + SSIM_C2
    QS = {"x": 0, "y": 1, "w": 2, "s": 3}   # stream order in yv blocks
    N_ACC = N_PLANES * NWIN                 # total acc matmuls (30)

    with tile.TileContext(nc) as tc:
        with (
            tc.tile_pool(name="const_p", bufs=1) as const_p,
            tc.tile_pool(name="in16_p", bufs=2) as in16_p,
            tc.tile_pool(name="pre_p", bufs=2) as pre_p,
            tc.tile_pool(name="yv_p", bufs=2) as yv_p,
            tc.tile_pool(name="post_p", bufs=2) as post_p,
            tc.tile_pool(name="ps1_p", bufs=1, space="PSUM") as ps1_p,
            tc.tile_pool(name="ps2_p", bufs=1, space="PSUM") as ps2_p,
            tc.tile_pool(name="acc_p", bufs=1, space="PSUM") as acc_p,
        ):
            band_t = const_p.tile([128, NWIN * 128], FP16, name="band_t",
                                  tag="band_t")
            bandn_t = const_p.tile([128, NWIN * 128], FP16, name="bandn_t",
                                   tag="bandn_t")
            band2_t = const_p.tile([128, NWIN * 128], FP16, name="band2_t",
                                   tag="band2_t")
            nc.sync.dma_start(out=band_t[:], in_=bands[:, :])
            nc.sync.dma_start(out=bandn_t[:], in_=bandsn[:, :])
            nc.sync.dma_start(out=band2_t[:], in_=bands2[:, :])

            ones_t = const_p.tile([128, 1], FP16, name="ones_t", tag="ones_t")
            nc.vector.memset(ones_t[:], 1.0)

            acc_ps = acc_p.tile([1, 512], FP32, name="acc_ps")

            # --- fp16 input loads (hardware DMA), one per plane/tensor,
            # issued ~two planes ahead from the pipeline loop ---
            x16 = {}
            y16 = {}

            def emit_load(p, half=None):
                if half in (None, 0):
                    x16[p] = in16_p.tile([128, WF], FP16, name="x16",
                                         tag="x16")
                    y16[p] = in16_p.tile([128, WF], FP16, name="y16",
                                         tag="y16")
                if half is None:
                    w0, wn = 0, W
                else:
                    w0, wn = half * (W // 2), W // 2
                for dst, src, eng in ((x16[p], img1, nc.sync),
                                      (y16[p], img2, nc.scalar)):
                    ap = bass.AP(src, p * H * W + w0,
                                 [[W, 128], [STRIDE * W, NWIN], [1, wn]])
                    eng.dma_start(
                        out=dst.rearrange("pt (c w) -> pt c w",
                                          c=NWIN)[:, :, w0:w0 + wn],
                        in_=ap)

            pre = {}     # plane -> dict of u/v/p/q stream tiles (fp16)

            def emit_pre(p, half=None):
                if half in (None, 0):
                    w16 = pre_p.tile([128, WF], FP16, name="w16", tag="w16")
                    sx16 = pre_p.tile([128, WF], FP16, name="sx16",
                                      tag="sx16")
                    sy16 = pre_p.tile([128, WF], FP16, name="sy16",
                                      tag="sy16")
                    pre[p] = {"w": w16, "sx": sx16, "sy": sy16}
                w16 = pre[p]["w"]
                sx16 = pre[p]["sx"]
                sy16 = pre[p]["sy"]
                if half is None:
                    sl = [slice(None)]
                else:
                    w0 = half * (W // 2)
                    sl3 = (slice(None), slice(None), slice(w0, w0 + W // 2))
                xv = x16[p].rearrange("pt (c w) -> pt c w", c=NWIN)
                yv_ = y16[p].rearrange("pt (c w) -> pt c w", c=NWIN)
                wv = w16.rearrange("pt (c w) -> pt c w", c=NWIN)
                sxv = sx16.rearrange("pt (c w) -> pt c w", c=NWIN)
                syv = sy16.rearrange("pt (c w) -> pt c w", c=NWIN)
                if half is None:
                    nc.vector.tensor_tensor(w16[:], x16[p][:], y16[p][:],
                                            Alu.mult)
                    nc.vector.tensor_tensor(sx16[:], x16[p][:], x16[p][:],
                                            Alu.mult)
                    nc.vector.tensor_tensor(sy16[:], y16[p][:], y16[p][:],
                                            Alu.mult)
                else:
                    nc.vector.tensor_tensor(wv[sl3], xv[sl3], yv_[sl3],
                                            Alu.mult)
                    nc.vector.tensor_tensor(sxv[sl3], xv[sl3], xv[sl3],
                                            Alu.mult)
                    nc.vector.tensor_tensor(syv[sl3], yv_[sl3], yv_[sl3],
                                            Alu.mult)

            yv = {}      # plane -> [128, 5*2048] f16 (per cw: u|v|p|q 512-blocks)

            def emit_pass1(p, cw):
                if cw == 0:
                    yv[p] = yv_p.tile([128, NWIN * 2048], FP16, name="yv",
                                      tag="yv")
                pr = pre[p]
                # stream-major: each stream's matmuls then its drain copy,
                # rotating over 3 PSUM banks.
                dve_drains = ()
                srcs = {"x": (x16[p],), "y": (y16[p],), "w": (pr["w"],),
                        "s": (pr["sx"], pr["sy"])}
                for qn in ("x", "y", "w", "s"):
                    p1 = ps1_p.tile([128, 512], FP32, name=f"p1{qn}", tag="p1",
                                    bufs=3)
                    bsrc = band2_t if qn == "w" else band_t
                    for c, (s, n) in enumerate(CHUNKS):
                        col = c * 512 + STRIDE * cw
                        bnd = bsrc[:, c * 128:c * 128 + n]
                        parts = srcs[qn]
                        for pi, ps in enumerate(parts):
                            nc.tensor.matmul(
                                p1[:, s:s + n], ps[:, col:col + 128],
                                bnd, start=(pi == 0),
                                stop=(pi == len(parts) - 1))
                    dst = yv[p][:, cw * 2048 + QS[qn] * 512:
                                cw * 2048 + QS[qn] * 512 + 512]
                    if qn in dve_drains:
                        nc.vector.tensor_copy(dst, p1[:, :])
                    else:
                        nc.scalar.copy(dst, p1[:, :])

            acc_n = [0]

            def emit_pass2_post(p, c2):
                s2, n2 = CHUNKS[c2]
                bnd = band_t[:, c2 * 128:c2 * 128 + n2]
                bndn = bandn_t[:, c2 * 128:c2 * 128 + n2]
                mamb = ps2_p.tile([128, 1024], FP32, name="mamb", tag="mamb")
                cdm = ps2_p.tile([128, 512], FP32, name="cdm", tag="cdm")
                csm = ps2_p.tile([128, 512], FP32, name="csm", tag="csm")
                yvx = yv[p][:, c2 * 2048 + QS["x"] * 512:c2 * 2048 + QS["x"] * 512 + 512]
                yvy = yv[p][:, c2 * 2048 + QS["y"] * 512:c2 * 2048 + QS["y"] * 512 + 512]
                yvw = yv[p][:, c2 * 2048 + QS["w"] * 512:c2 * 2048 + QS["w"] * 512 + 512]
                yvs = yv[p][:, c2 * 2048 + QS["s"] * 512:c2 * 2048 + QS["s"] * 512 + 512]
                nc.tensor.matmul(mamb[:n2, 0:512], bnd, yvx, start=True, stop=False)
                nc.tensor.matmul(mamb[:n2, 0:512], bnd, yvy, start=False, stop=True)
                nc.tensor.matmul(mamb[:n2, 512:1024], bnd, yvx, start=True, stop=False)
                nc.tensor.matmul(mamb[:n2, 512:1024], bndn, yvy, start=False, stop=True)
                nc.tensor.matmul(cdm[:n2, :], bnd, yvw, start=True, stop=True)
                nc.tensor.matmul(csm[:n2, :], bnd, yvs, start=True, stop=True)

                ABt = post_p.tile([128, 1024], FP16, name="ABt", tag="ABt")
                r2t = post_p.tile([128, 512], FP16, name="r2t", tag="r2t")
                msqt = post_p.tile([128, 512], FP16, name="msqt", tag="msqt")
                numt = post_p.tile([128, 512], FP16, name="numt", tag="numt")
                dent = post_p.tile([128, 512], FP32, name="dent", tag="dent")
                rect = post_p.tile([128, 512], FP16, name="rect", tag="rect")
                scr = post_p.tile([128, 512], FP16, name="scr", tag="scr",
                                  bufs=3)
                nc.scalar.activation(ABt[:n2, :], mamb[:n2, :], Act.Square,
                                     scale=INV_SQRT2)
                nc.vector.tensor_tensor(
                    r2t[:n2, :], ABt[:n2, 0:512], ABt[:n2, 512:1024],
                    Alu.subtract)
                nc.vector.tensor_tensor(
                    msqt[:n2, :], ABt[:n2, 0:512], ABt[:n2, 512:1024],
                    Alu.add)
                nc.vector._custom_dve(
                    ops["nd"], out=numt[:n2, :], in0=cdm[:n2, :],
                    in1=r2t[:n2, :], s0=SSIM_C2, s1=SSIM_C1)
                nc.vector._custom_dve(
                    ops["nd"], out=dent[:n2, :], in0=csm[:n2, :],
                    in1=msqt[:n2, :], s0=SSIM_C2, s1=SSIM_C1)
                nc.vector._custom_dve(
                    ops["recip"], out=rect[:n2, :], in0=dent[:n2, :],
                    s0=rc["s0"], s1=rc["s1"], imm2=rc["imm2"])
                nc.vector.tensor_tensor(
                    scr[:n2, :], numt[:n2, :], rect[:n2, :], Alu.mult)
                acc_q.append((scr, n2))

            def emit_acc():
                scr, n2 = acc_q.popleft()
                k = acc_n[0]
                acc_n[0] += 1
                nc.tensor.matmul(acc_ps[:, :], ones_t[:n2, :], scr[:n2, :],
                                 start=(k == 0), stop=(k == N_ACC - 1))

            from collections import deque
            pending = deque()
            acc_q = deque()
            emit_load(0)
            emit_load(1)
            for p in range(N_PLANES):
                for cw in range(NWIN):
                    if p == 0 and cw == 0:
                        emit_pre(0)
                    emit_pass1(p, cw)
                    pending.append((p, cw))
                    lag = 2 if p == N_PLANES - 1 and cw >= 3 else 2
                    if len(pending) > lag:
                        emit_pass2_post(*pending.popleft())
                    if len(acc_q) > 1:
                        emit_acc()
                    if cw == 1 and p + 2 < N_PLANES:
                        emit_load(p + 2)
                    if cw == 1 and p + 1 < N_PLANES:
                        emit_pre(p + 1)
            while pending:
                emit_pass2_post(*pending.popleft())
            while acc_q:
                emit_acc()

            out_sb = const_p.tile([1, 512], FP32, name="out_sb", tag="out_sb")
            nc.vector.tensor_copy(out_sb[:, :], acc_ps[:, :])
            nc.sync.dma_start(out=out[:, :], in_=out_sb[:, :])

    nc.compile()
    return nc


_NC_CACHE = None


def _in_maps(img1, img2):
    img1 = np.ascontiguousarray(img1, dtype=np.float32).astype(np.float16)
    img2 = np.ascontiguousarray(img2, dtype=np.float32).astype(np.float16)
    bands = _build_bands()
    return [
        {
            "img1": img1[i * B_LOCAL:(i + 1) * B_LOCAL],
            "img2": img2[i * B_LOCAL:(i + 1) * B_LOCAL],
            "bands": bands,
            "bandsn": (-bands.astype(np.float32)).astype(np.float16),
            "bands2": (bands.astype(np.float32) * 2.0).astype(np.float16),
        }
        for i in range(N_CORES)
    ]


def kernel(img1: np.ndarray, img2: np.ndarray) -> np.ndarray:
    global _NC_CACHE
    if _NC_CACHE is None:
        _NC_CACHE = _build_graph()
    nc = _NC_CACHE

    res = run_bass_kernel_spmd(nc, _in_maps(img1, img2), list(range(N_CORES)))
    total = np.float64(0.0)
    for r in res.results:
        total += np.asarray(r["out"], dtype=np.float64).sum()
    mean = total / (B_FULL * C * H * W)
    return np.array(mean, dtype=np.float32)


# revision 35
# speedup vs baseline: 1.0814x; 1.0455x over previous
"""SSIM loss kernel for Trainium2, SPMD over 8 NeuronCores.

Inputs: img1, img2 [16,3,512,512] f32. Output: scalar mean SSIM (f32).
Sharding: batch dim 16 -> 2 per core; host sums per-core partial sums.

Math (per pixel, 11x11 Gaussian window, C1=1e-4, C2=9e-4), u/v basis:
  u = x+y, v = x-y; streams u, v, p=u^2/2, q=v^2/2
  ma = conv(u) = mu1+mu2, mb = conv(v) = mu1-mu2
  cdm = conv(p) - conv(q) = 2*conv(xy) = cd
  csm = conv(p) + conv(q) = conv(x^2)+conv(y^2) = cs
  A = ma^2/2, B = mb^2/2; r2c1 = (A+C1)-B = 2 mu1 mu2 + C1
  mqc1 = (A+C1)+B = mu1^2+mu2^2 + C1
  num = (cdm - r2c1 + C1+C2) * r2c1   [custom DVE nd2]
  den = (csm - mqc1 + C1+C2) * mqc1   [custom DVE nd2]
  ssim = num * recip(den); mean via PE ones-matmul reduction.

Implementation notes:
- Inputs loaded fp32 by hardware DMA (sync + scalar HWDGE queues, one
  tensor each), cast to fp16 by ACT copies; u/v adds on DVE fp16 TT (2x).
- 11-tap separable conv as dense band matmuls: pass 1 convolves H
  data-stationary (output transposed, W on partitions; 5 overlapping
  128-row windows, stride 96); pass 2 convolves W band-stationary.
- Pass-1 emission stream-major with per-stream PSUM->SBUF drain
  (ACT/DVE alternating); PSUM: pass1 rotates 3 banks, pass2 4, acc 1.
- Per-pixel SSIM summed on PE: acc_ps[1,512] += ones^T @ scr, one
  accumulation group across the whole kernel; host sums the 512 lanes.
- fp16 band taps ulp-nudged so sum(fl16(g)) == 1 (sigma paths are
  first-order sensitive to the weight sum).
- Software-pipelined emission (2 window-slot stagger) keeps PE warm.
"""

import math

import numpy as np

from concourse import bacc, bass, mybir, tile
from concourse.bass_utils import run_bass_kernel_spmd

B_FULL, C, H, W = 16, 3, 512, 512
N_CORES = 8
B_LOCAL = B_FULL // N_CORES          # 2
N_PLANES = B_LOCAL * C               # 6 spatial planes per core
KSZ = 11
PAD = KSZ // 2
SSIM_C1 = 0.01 ** 2
SSIM_C2 = 0.03 ** 2

STRIDE = 96
NWIN = 5
CHUNKS = [(0, 101), (101, 96), (197, 96), (293, 96), (389, 123)]
WF = NWIN * 512                      # 2560 free cols per plane

FP32 = mybir.dt.float32
FP16 = mybir.dt.float16

_OPS = {}


def _register_custom_ops():
    """Idempotently register the SSIM custom DVE ops."""
    global _OPS
    if _OPS:
        return _OPS
    import concourse.dve_ops as D
    from concourse.dve_spec import Spec, Src0, Src1, C0, C1, lower, _has_src1
    from concourse.dve_uop import DveOpSpec

    def reg(op):
        D.OPS.append(op)
        D._SUB_OPCODE_FOR_NAME[op.name] = D._CUSTOM_DVE_ROW_BASE + len(D.OPS) - 1
        D.CUSTOM_DVE_SPECS[op.name] = op.spec
        for ver in ("v3", "v4"):
            uops = lower(op.spec, ver=ver)
            so = DveOpSpec(name=op.name, opcode=D.get_dve_sub_opcode(op.name),
                           uops=uops, rd1_en=_has_src1(op.spec))
            op.uops_sha[ver] = so.sha(ver)
        return op

    if "SSIM_ND_ANT" in D._SUB_OPCODE_FOR_NAME:
        nd = next(o for o in D.OPS if o.name == "SSIM_ND_ANT")
    else:
        nd = reg(D.DveOp(
            "SSIM_ND_ANT",
            Spec(body=(Src0 - Src1 + C0) * (Src1 + C1),
                 reference=lambda in0, in1, s0, s1, imm2:
                     (in0.astype(np.float32) - in1 + s0)
                     * (in1.astype(np.float32) + s1)),
            subdim=False, uops_sha={}))
    _OPS = {"nd": nd,
            "recip": D.RECIPROCAL_APPROX_FAST,
            "recip_consts": D.RECIP_APPROX_FAST_CONSTS}
    return _OPS


def _gaussian_1d():
    x = np.arange(KSZ)
    g = np.exp(-((x - KSZ // 2) ** 2) / (2.0 * 1.5 ** 2))
    return (g / g.sum()).astype(np.float64)


def _gaussian_1d_f16():
    """fp16 taps nudged by +-1 ulp so sum(fl16(g)) == 1 to ~1e-7."""
    g16 = _gaussian_1d().astype(np.float16)
    for _ in range(200):
        e = g16.astype(np.float64).sum() - 1.0
        if abs(e) < 5e-8:
            break
        best = None
        for i in range(KSZ):
            step = np.nextafter(g16[i], np.float16(1.0 if e < 0 else 0.0))
            ne = e + (float(step) - float(g16[i]))
            if best is None or abs(ne) < abs(best[1]):
                best = (i, ne, step)
        i, ne, step = best
        if abs(ne) >= abs(e):
            break
        g16[i] = step
    return g16.astype(np.float64)


def _build_bands():
    """[128, 5*128] f16; window c at cols [128c, 128c+n_c).
    out[s+jj] = sum_r band[r, 128c+jj] * x[96c + r]."""
    g = _gaussian_1d_f16()
    bands = np.zeros((128, NWIN * 128), dtype=np.float64)
    for c, (s, n) in enumerate(CHUNKS):
        r0 = STRIDE * c
        for r in range(128):
            for jj in range(n):
                t = (r0 + r) - (s + jj) + PAD
                if 0 <= t < KSZ:
                    bands[r, c * 128 + jj] = g[t]
    return bands.astype(np.float16)


def _build_graph():
    ops = _register_custom_ops()
    nc = bacc.Bacc()
    img1 = nc.declare_dram_parameter("img1", [B_LOCAL, C, H, W], FP16, isOutput=False)
    img2 = nc.declare_dram_parameter("img2", [B_LOCAL, C, H, W], FP16, isOutput=False)
    bands = nc.declare_dram_parameter("bands", [128, NWIN * 128], FP16, isOutput=False)
    bandsn = nc.declare_dram_parameter("bandsn", [128, NWIN * 128], FP16, isOutput=False)
    bands2 = nc.declare_dram_parameter("bands2", [128, NWIN * 128], FP16, isOutput=False)
    out = nc.declare_dram_parameter("out", [1, 512], FP32, isOutput=True)

    Alu = mybir.AluOpType
    Act = mybir.ActivationFunctionType
    rc = ops["recip_consts"]
    INV_SQRT2 = 1.0 / math.sqrt(2.0)
    C12 = SSIM_C1 + SSIM_C2
    QS = {"x": 0, "y": 1, "w": 2, "s": 3}   # stream order in yv blocks
    N_ACC = N_PLANES * NWIN                 # total acc matmuls (30)

    with tile.TileContext(nc) as tc:
        with (
            tc.tile_pool(name="const_p", bufs=1) as const_p,
            tc.tile_pool(name="in16_p", bufs=3) as in16_p,
            tc.tile_pool(name="pre_p", bufs=2) as pre_p,
            tc.tile_pool(name="yv_p", bufs=3) as yv_p,
            tc.tile_pool(name="post_p", bufs=2) as post_p,
            tc.tile_pool(name="ps1_p", bufs=1, space="PSUM") as ps1_p,
            tc.tile_pool(name="ps2_p", bufs=1, space="PSUM") as ps2_p,
            tc.tile_pool(name="acc_p", bufs=1, space="PSUM") as acc_p,
        ):
            band_t = const_p.tile([128, NWIN * 128], FP16, name="band_t",
                                  tag="band_t")
            bandn_t = const_p.tile([128, NWIN * 128], FP16, name="bandn_t",
                                   tag="bandn_t")
            band2_t = const_p.tile([128, NWIN * 128], FP16, name="band2_t",
                                   tag="band2_t")
            nc.sync.dma_start(out=band_t[:], in_=bands[:, :])
            nc.sync.dma_start(out=bandn_t[:], in_=bandsn[:, :])
            nc.sync.dma_start(out=band2_t[:], in_=bands2[:, :])

            ones_t = const_p.tile([128, 1], FP16, name="ones_t", tag="ones_t")
            nc.vector.memset(ones_t[:], 1.0)

            acc_ps = acc_p.tile([1, 512], FP32, name="acc_ps")

            # --- fp16 input loads (hardware DMA), one per plane/tensor,
            # issued ~two planes ahead from the pipeline loop ---
            x16 = {}
            y16 = {}

            def emit_load(p, half=None):
                if half in (None, 0):
                    x16[p] = in16_p.tile([128, WF], FP16, name="x16",
                                         tag="x16")
                    y16[p] = in16_p.tile([128, WF], FP16, name="y16",
                                         tag="y16")
                if half is None:
                    w0, wn = 0, W
                else:
                    w0, wn = half * (W // 2), W // 2
                for dst, src, eng in ((x16[p], img1, nc.sync),
                                      (y16[p], img2, nc.scalar)):
                    ap = bass.AP(src, p * H * W + w0,
                                 [[W, 128], [STRIDE * W, NWIN], [1, wn]])
                    eng.dma_start(
                        out=dst.rearrange("pt (c w) -> pt c w",
                                          c=NWIN)[:, :, w0:w0 + wn],
                        in_=ap)

            pre = {}     # plane -> dict of u/v/p/q stream tiles (fp16)

            def emit_pre(p, half=None):
                if half in (None, 0):
                    w16 = pre_p.tile([128, WF], FP16, name="w16", tag="w16")
                    sx16 = pre_p.tile([128, WF], FP16, name="sx16",
                                      tag="sx16")
                    sy16 = pre_p.tile([128, WF], FP16, name="sy16",
                                      tag="sy16")
                    pre[p] = {"w": w16, "sx": sx16, "sy": sy16}
                w16 = pre[p]["w"]
                sx16 = pre[p]["sx"]
                sy16 = pre[p]["sy"]
                if half is None:
                    sl = [slice(None)]
                else:
                    w0 = half * (W // 2)
                    sl3 = (slice(None), slice(None), slice(w0, w0 + W // 2))
                xv = x16[p].rearrange("pt (c w) -> pt c w", c=NWIN)
                yv_ = y16[p].rearrange("pt (c w) -> pt c w", c=NWIN)
                wv = w16.rearrange("pt (c w) -> pt c w", c=NWIN)
                sxv = sx16.rearrange("pt (c w) -> pt c w", c=NWIN)
                syv = sy16.rearrange("pt (c w) -> pt c w", c=NWIN)
                if half is None:
                    nc.vector.tensor_tensor(w16[:], x16[p][:], y16[p][:],
                                            Alu.mult)
                    nc.vector.tensor_tensor(sx16[:], x16[p][:], x16[p][:],
                                            Alu.mult)
                    nc.vector.tensor_tensor(sy16[:], y16[p][:], y16[p][:],
                                            Alu.mult)
                else:
                    nc.vector.tensor_tensor(wv[sl3], xv[sl3], yv_[sl3],
                                            Alu.mult)
                    nc.vector.tensor_tensor(sxv[sl3], xv[sl3], xv[sl3],
                                            Alu.mult)
                    nc.vector.tensor_tensor(syv[sl3], yv_[sl3], yv_[sl3],
                                            Alu.mult)

            yv = {}      # plane -> [128, 5*2048] f16 (per cw: u|v|p|q 512-blocks)

            def emit_pass1(p, cw):
                if cw == 0:
                    yv[p] = yv_p.tile([128, NWIN * 2048], FP16, name="yv",
                                      tag="yv")
                pr = pre[p]
                # stream-major: each stream's matmuls then its drain copy,
                # rotating over 3 PSUM banks.
                dve_drains = ()
                srcs = {"x": (x16[p],), "y": (y16[p],), "w": (pr["w"],),
                        "s": (pr["sx"], pr["sy"])}
                for qn in ("x", "y", "w", "s"):
                    p1 = ps1_p.tile([128, 512], FP32, name=f"p1{qn}", tag="p1",
                                    bufs=3)
                    bsrc = band2_t if qn == "w" else band_t
                    for c, (s, n) in enumerate(CHUNKS):
                        col = c * 512 + STRIDE * cw
                        bnd = bsrc[:, c * 128:c * 128 + n]
                        parts = srcs[qn]
                        for pi, ps in enumerate(parts):
                            nc.tensor.matmul(
                                p1[:, s:s + n], ps[:, col:col + 128],
                                bnd, start=(pi == 0),
                                stop=(pi == len(parts) - 1))
                    dst = yv[p][:, cw * 2048 + QS[qn] * 512:
                                cw * 2048 + QS[qn] * 512 + 512]
                    if qn in dve_drains:
                        nc.vector.tensor_copy(dst, p1[:, :])
                    else:
                        nc.scalar.copy(dst, p1[:, :])

            acc_n = [0]

            def emit_pass2_post(p, c2):
                s2, n2 = CHUNKS[c2]
                bnd = band_t[:, c2 * 128:c2 * 128 + n2]
                bndn = bandn_t[:, c2 * 128:c2 * 128 + n2]
                mamb = ps2_p.tile([128, 1024], FP32, name="mamb", tag="mamb")
                cdm = ps2_p.tile([128, 512], FP32, name="cdm", tag="cdm")
                csm = ps2_p.tile([128, 512], FP32, name="csm", tag="csm")
                yvx = yv[p][:, c2 * 2048 + QS["x"] * 512:c2 * 2048 + QS["x"] * 512 + 512]
                yvy = yv[p][:, c2 * 2048 + QS["y"] * 512:c2 * 2048 + QS["y"] * 512 + 512]
                yvw = yv[p][:, c2 * 2048 + QS["w"] * 512:c2 * 2048 + QS["w"] * 512 + 512]
                yvs = yv[p][:, c2 * 2048 + QS["s"] * 512:c2 * 2048 + QS["s"] * 512 + 512]
                nc.tensor.matmul(mamb[:n2, 0:512], bnd, yvx, start=True, stop=False)
                nc.tensor.matmul(mamb[:n2, 0:512], bnd, yvy, start=False, stop=True)
                nc.tensor.matmul(mamb[:n2, 512:1024], bnd, yvx, start=True, stop=False)
                nc.tensor.matmul(mamb[:n2, 512:1024], bndn, yvy, start=False, stop=True)
                nc.tensor.matmul(cdm[:n2, :], bnd, yvw, start=True, stop=True)
                nc.tensor.matmul(csm[:n2, :], bnd, yvs, start=True, stop=True)

                ABt = post_p.tile([128, 1024], FP16, name="ABt", tag="ABt")
                r2t = post_p.tile([128, 512], FP16, name="r2t", tag="r2t")
                msqt = post_p.tile([128, 512], FP16, name="msqt", tag="msqt")
                numt = post_p.tile([128, 512], FP16, name="numt", tag="numt")
                dent = post_p.tile([128, 512], FP32, name="dent", tag="dent")
                rect = post_p.tile([128, 512], FP16, name="rect", tag="rect")
                scr = post_p.tile([128, 512], FP16, name="scr", tag="scr",
                                  bufs=3)
                nc.scalar.activation(ABt[:n2, :], mamb[:n2, :], Act.Square,
                                     scale=INV_SQRT2)
                nc.vector.tensor_tensor(
                    r2t[:n2, :], ABt[:n2, 0:512], ABt[:n2, 512:1024],
                    Alu.subtract)
                nc.vector.tensor_tensor(
                    msqt[:n2, :], ABt[:n2, 0:512], ABt[:n2, 512:1024],
                    Alu.add)
                nc.vector._custom_dve(
                    ops["nd"], out=numt[:n2, :], in0=cdm[:n2, :],
                    in1=r2t[:n2, :], s0=SSIM_C2, s1=SSIM_C1)
                nc.vector._custom_dve(
                    ops["nd"], out=dent[:n2, :], in0=csm[:n2, :],
                    in1=msqt[:n2, :], s0=SSIM_C2, s1=SSIM_C1)
                nc.vector._custom_dve(
                    ops["recip"], out=rect[:n2, :], in0=dent[:n2, :],
                    s0=rc["s0"], s1=rc["s1"], imm2=rc["imm2"])
                nc.vector.tensor_tensor(
                    scr[:n2, :], numt[:n2, :], rect[:n2, :], Alu.mult)
                acc_q.append((scr, n2))

            def emit_acc():
                scr, n2 = acc_q.popleft()
                k = acc_n[0]
                acc_n[0] += 1
                nc.tensor.matmul(acc_ps[:, :], ones_t[:n2, :], scr[:n2, :],
                                 start=(k == 0), stop=(k == N_ACC - 1))

            from collections import deque
            pending = deque()
            acc_q = deque()
            emit_load(0)
            emit_load(1)
            for p in range(N_PLANES):
                for cw in range(NWIN):
                    if p == 0 and cw == 0:
                        emit_pre(0)
                    emit_pass1(p, cw)
                    pending.append((p, cw))
                    lag = 2 if p == N_PLANES - 1 and cw >= 3 else 2
                    if len(pending) > lag:
                        emit_pass2_post(*pending.popleft())
                    if len(acc_q) > 1:
                        emit_acc()
                    if cw == 1 and p + 2 < N_PLANES:
                        emit_load(p + 2)
                    if cw == 1 and p + 1 < N_PLANES:
                        emit_pre(p + 1)
            while pending:
                emit_pass2_post(*pending.popleft())
            while acc_q:
                emit_acc()

            out_sb = const_p.tile([1, 512], FP32, name="out_sb", tag="out_sb")
            nc.vector.tensor_copy(out_sb[:, :], acc_ps[:, :])
            nc.sync.dma_start(out=out[:, :], in_=out_sb[:, :])

    nc.compile()
    return nc


_NC_CACHE = None


def _in_maps(img1, img2):
    img1 = np.ascontiguousarray(img1, dtype=np.float32).astype(np.float16)
    img2 = np.ascontiguousarray(img2, dtype=np.float32).astype(np.float16)
    bands = _build_bands()
    return [
        {
            "img1": img1[i * B_LOCAL:(i + 1) * B_LOCAL],
            "img2": img2[i * B_LOCAL:(i + 1) * B_LOCAL],
            "bands": bands,
            "bandsn": (-bands.astype(np.float32)).astype(np.float16),
            "bands2": (bands.astype(np.float32) * 2.0).astype(np.float16),
        }
        for i in range(N_CORES)
    ]


def kernel(img1: np.ndarray, img2: np.ndarray) -> np.ndarray:
    global _NC_CACHE
    if _NC_CACHE is None:
        _NC_CACHE = _build_graph()
    nc = _NC_CACHE

    res = run_bass_kernel_spmd(nc, _in_maps(img1, img2), list(range(N_CORES)))
    total = np.float64(0.0)
    for r in res.results:
        total += np.asarray(r["out"], dtype=np.float64).sum()
    mean = total / (B_FULL * C * H * W)
    return np.array(mean, dtype=np.float32)
